# revision 2
# baseline (speedup 1.0000x reference)
"""AttentionHead kernel for 8 TRN2 NeuronCores.

Reference computes (no softmax!):
    Q = x @ qW^T + qb ; K = x @ kW^T + kb ; V = x @ vW^T + vb    [B,S,E]
    out = (Q @ K^T * scale) @ V                                  [B,S,E]

Since there is no softmax, matmul associativity gives
    out = Q @ (scale * K^T V)
where M := scale * K^T V is only [E,E] = [128,128].  This removes the
S x S score matrix entirely: ~23.6 GFLOP -> ~7 GFLOP.

Sharding: rows (B*S = 8192) split into 8 contiguous shards of 1024 rows
(cores 0-3 = batch 0, cores 4-7 = batch 1).  Each core computes K/V
projections for its rows, accumulates a partial M, AllGathers the 4
partials within its batch group, sums them, and applies O = Q @ M to its
rows.  Weights are replicated; x is fed pre-transposed (x^T) so the
contraction dim (d) lands on SBUF partitions with no on-device
transposes of x.  Compute dtype bf16 (f32 PSUM accumulation).
"""

import numpy as np
import ml_dtypes
from contextlib import ExitStack

import concourse.bass as bass
import concourse.tile as tile
import concourse.mybir as mybir
from concourse import bacc
from concourse.bass_utils import run_bass_kernel_spmd

# Problem shape (hardcoded per harness contract)
B, S, D, E = 2, 4096, 1024, 128
NCORES = 8
RPC = B * S // NCORES          # rows per core = 1024
DC = D // 128                  # d-chunks = 8
RC8 = RPC // 128               # 128-row chunks per core = 8
SCALE = 1.0 / float(np.sqrt(E))
GROUPS = [[0, 1, 2, 3], [4, 5, 6, 7]]

F32 = mybir.dt.float32
BF16 = mybir.dt.bfloat16


def _build_graph():
    nc = bacc.Bacc(
        "TRN2",
        target_bir_lowering=False,
        debug=False,
        enable_asserts=False,
        num_devices=NCORES,
    )

    # Kernel I/O (per-core shards / replicated weights)
    xT = nc.dram_tensor("xT", [D, RPC], BF16, kind="ExternalInput")
    qWT = nc.dram_tensor("qWT", [D, E], BF16, kind="ExternalInput")
    kWT = nc.dram_tensor("kWT", [D, E], BF16, kind="ExternalInput")
    vWT = nc.dram_tensor("vWT", [D, E], BF16, kind="ExternalInput")
    qb = nc.dram_tensor("qb", [E, 1], F32, kind="ExternalInput")
    kb = nc.dram_tensor("kb", [E, 1], F32, kind="ExternalInput")
    vb = nc.dram_tensor("vb", [E, 1], F32, kind="ExternalInput")
    ident = nc.dram_tensor("ident", [128, 128], BF16, kind="ExternalInput")
    out = nc.dram_tensor("out", [RPC, E], F32, kind="ExternalOutput")

    # Internal bounce buffers for the collective
    cc_in = nc.dram_tensor("cc_in", [E, E], F32)
    cc_out = nc.dram_tensor("cc_out", [4 * E, E], F32)

    with tile.TileContext(nc) as tc, ExitStack() as ctx:
        consts = ctx.enter_context(tc.tile_pool(name="consts", bufs=1))
        proj_sb = ctx.enter_context(tc.tile_pool(name="proj_sb", bufs=1))
        kv_sb = ctx.enter_context(tc.tile_pool(name="kv_sb", bufs=1))
        m_pool = ctx.enter_context(tc.tile_pool(name="m_pool", bufs=1))
        o_pool = ctx.enter_context(tc.tile_pool(name="o_pool", bufs=2))
        ps_proj = ctx.enter_context(tc.tile_pool(name="ps_proj", bufs=2, space="PSUM"))
        ps_tr = ctx.enter_context(tc.tile_pool(name="ps_tr", bufs=2, space="PSUM"))
        ps_m = ctx.enter_context(tc.tile_pool(name="ps_m", bufs=1, space="PSUM"))
        ps_o = ctx.enter_context(tc.tile_pool(name="ps_o", bufs=2, space="PSUM"))

        # ---- constant / input loads ----
        id_t = consts.tile([128, 128], BF16, tag="ident")
        nc.sync.dma_start(id_t[:, :], ident[:, :])

        wk_t = consts.tile([128, DC * E], BF16, tag="wk")
        wv_t = consts.tile([128, DC * E], BF16, tag="wv")
        wq_t = consts.tile([128, DC * E], BF16, tag="wq")
        # weight DRAM [(c p), e] -> SBUF [p, c, e]
        nc.sync.dma_start(
            wk_t[:, :].rearrange("p (c e) -> p c e", c=DC),
            kWT.ap().rearrange("(c p) e -> p c e", p=128),
        )
        nc.sync.dma_start(
            wv_t[:, :].rearrange("p (c e) -> p c e", c=DC),
            vWT.ap().rearrange("(c p) e -> p c e", p=128),
        )
        nc.sync.dma_start(
            wq_t[:, :].rearrange("p (c e) -> p c e", c=DC),
            qWT.ap().rearrange("(c p) e -> p c e", p=128),
        )

        qb_t = consts.tile([E, 1], F32, tag="qb")
        kb_t = consts.tile([E, 1], F32, tag="kb")
        vb_t = consts.tile([E, 1], F32, tag="vb")
        nc.sync.dma_start(qb_t[:, :], qb[:, :])
        nc.sync.dma_start(kb_t[:, :], kb[:, :])
        nc.sync.dma_start(vb_t[:, :], vb[:, :])

        # x^T: 8 chunks of [128(d), RPC(r)]
        xt_t = consts.tile([128, DC * RPC], BF16, tag="xt")
        for d in range(DC):
            nc.sync.dma_start(
                xt_t[:, d * RPC : (d + 1) * RPC],
                xT[d * 128 : (d + 1) * 128, :],
            )

        def xt(d, r0, rn):
            return xt_t[:, d * RPC + r0 : d * RPC + r0 + rn]

        # ---- K^T / V^T projections: [e, r] = sum_d W^T[d,e].T @ x^T[d,r] ----
        KT = proj_sb.tile([128, RPC], BF16, tag="KT")
        VT = proj_sb.tile([128, RPC], BF16, tag="VT")
        NPR = 512  # moving free dim per matmul
        for w_t, b_t, dst in ((wk_t, kb_t, KT), (wv_t, vb_t, VT)):
            for r0 in range(0, RPC, NPR):
                ps = ps_proj.tile([128, NPR], F32, tag="ps_proj")
                for d in range(DC):
                    nc.tensor.matmul(
                        ps[:, :],
                        w_t[:, d * E : (d + 1) * E],
                        xt(d, r0, NPR),
                        start=(d == 0),
                        stop=(d == DC - 1),
                    )
                # bias + cast to bf16 on the way out of PSUM
                nc.scalar.activation(
                    dst[:, r0 : r0 + NPR],
                    ps[:, :],
                    mybir.ActivationFunctionType.Identity,
                    bias=b_t[:, :],
                    scale=1.0,
                )

        # ---- transpose K^T/V^T -> K,V [r, e] (PE transpose via identity) ----
        K_n = kv_sb.tile([128, RC8 * E], BF16, tag="K_n")
        V_n = kv_sb.tile([128, RC8 * E], BF16, tag="V_n")
        for src, dst in ((KT, K_n), (VT, V_n)):
            for r in range(RC8):
                pt = ps_tr.tile([128, 128], BF16, tag="ps_tr")
                nc.tensor.transpose(pt[:, :], src[:, r * 128 : (r + 1) * 128], id_t[:, :])
                nc.vector.tensor_copy(dst[:, r * E : (r + 1) * E], pt[:, :])

        # ---- M_local = scale * K^T V  (contract rows on partitions) ----
        mps = ps_m.tile([E, E], F32, tag="ps_m")
        for r in range(RC8):
            nc.tensor.matmul(
                mps[:, :],
                K_n[:, r * E : (r + 1) * E],
                V_n[:, r * E : (r + 1) * E],
                start=(r == 0),
                stop=(r == RC8 - 1),
            )
        M_loc = m_pool.tile([E, E], F32, tag="M_loc")
        nc.scalar.activation(
            M_loc[:, :],
            mps[:, :],
            mybir.ActivationFunctionType.Identity,
            bias=0.0,
            scale=SCALE,
        )

        # ---- AllGather partial Ms within each batch group of 4 cores ----
        nc.sync.dma_start(cc_in[:, :], M_loc[:, :])
        nc.gpsimd.collective_compute(
            "AllGather",
            mybir.AluOpType.bypass,
            replica_groups=GROUPS,
            ins=[cc_in.ap().opt()],
            outs=[cc_out.ap().opt()],
        )

        # ---- Q^T projection (overlaps the collective) ----
        QT = proj_sb.tile([128, RPC], BF16, tag="QT")
        for r0 in range(0, RPC, NPR):
            ps = ps_proj.tile([128, NPR], F32, tag="ps_proj")
            for d in range(DC):
                nc.tensor.matmul(
                    ps[:, :],
                    wq_t[:, d * E : (d + 1) * E],
                    xt(d, r0, NPR),
                    start=(d == 0),
                    stop=(d == DC - 1),
                )
            nc.scalar.activation(
                QT[:, r0 : r0 + NPR],
                ps[:, :],
                mybir.ActivationFunctionType.Identity,
                bias=qb_t[:, :],
                scale=1.0,
            )

        # ---- combine gathered Ms: M = sum of 4 partials, cast bf16 ----
        mg = m_pool.tile([128, 4 * E], F32, tag="mg")
        nc.sync.dma_start(
            mg[:, :].rearrange("p (c e) -> p c e", c=4),
            cc_out.ap().rearrange("(c p) e -> p c e", p=128),
        )
        t01 = m_pool.tile([E, E], F32, tag="t01")
        t23 = m_pool.tile([E, E], F32, tag="t23")
        msum = m_pool.tile([E, E], F32, tag="msum")
        m_bf = m_pool.tile([E, E], BF16, tag="m_bf")
        nc.vector.tensor_add(t01[:, :], mg[:, 0:E], mg[:, E : 2 * E])
        nc.vector.tensor_add(t23[:, :], mg[:, 2 * E : 3 * E], mg[:, 3 * E : 4 * E])
        nc.vector.tensor_add(msum[:, :], t01[:, :], t23[:, :])
        nc.vector.tensor_copy(m_bf[:, :], msum[:, :])

        # ---- O = Q @ M : out[r,e] = sum_e' Q^T[e',r] M[e',e] ----
        for r in range(RC8):
            po = ps_o.tile([128, E], F32, tag="ps_o")
            nc.tensor.matmul(
                po[:, :],
                QT[:, r * 128 : (r + 1) * 128],
                m_bf[:, :],
                start=True,
                stop=True,
            )
            o_sb = o_pool.tile([128, E], F32, tag="o_sb")
            nc.vector.tensor_copy(o_sb[:, :], po[:, :])
            nc.sync.dma_start(out[r * 128 : (r + 1) * 128, :], o_sb[:, :])

    nc.compile()
    return nc


_NC_CACHE = None


def _get_graph():
    global _NC_CACHE
    if _NC_CACHE is None:
        _NC_CACHE = _build_graph()
    return _NC_CACHE


def _prep_in_maps(x, qW_w, qW_b, kW_w, kW_b, vW_w, vW_b):
    bf = ml_dtypes.bfloat16
    xf = np.asarray(x, dtype=np.float32).reshape(B * S, D)
    qWT = np.ascontiguousarray(np.asarray(qW_w, dtype=np.float32).T).astype(bf)
    kWT = np.ascontiguousarray(np.asarray(kW_w, dtype=np.float32).T).astype(bf)
    vWT = np.ascontiguousarray(np.asarray(vW_w, dtype=np.float32).T).astype(bf)
    qbc = np.asarray(qW_b, dtype=np.float32).reshape(E, 1)
    kbc = np.asarray(kW_b, dtype=np.float32).reshape(E, 1)
    vbc = np.asarray(vW_b, dtype=np.float32).reshape(E, 1)
    ident = np.eye(128, dtype=np.float32).astype(bf)

    in_maps = []
    for c in range(NCORES):
        xs = np.ascontiguousarray(xf[c * RPC : (c + 1) * RPC].T).astype(bf)  # [D, RPC]
        in_maps.append(
            {
                "xT": xs,
                "qWT": qWT,
                "kWT": kWT,
                "vWT": vWT,
                "qb": qbc,
                "kb": kbc,
                "vb": vbc,
                "ident": ident,
            }
        )
    return in_maps


def run(inputs: dict, trace: bool = False):
    """Run on hardware; returns (full_output, BassKernelResults)."""
    nc = _get_graph()
    in_maps = _prep_in_maps(**inputs)
    res = run_bass_kernel_spmd(
        nc, in_maps, core_ids=list(range(NCORES)), trace=trace
    )
    shards = [np.asarray(res.results[c]["out"], dtype=np.float32) for c in range(NCORES)]
    full = np.concatenate(shards, axis=0).reshape(B, S, E)
    return full, res


def kernel(**inputs) -> np.ndarray:
    out, _ = run(inputs, trace=False)
    return out
